# revision 1
# baseline (speedup 1.0000x reference)
"""Trainium2 Bass kernel: Bahdanau local-p attention (B=32, S=2048, H=1024).

Sharding: data-parallel over batch. Each of the 8 cores processes B/8 = 4
batches end-to-end (weights replicated); no collectives.

Per-core dataflow (all matmul-heavy work in fp16 with fp32 PSUM accumulation):
  1. inputs[b] is cast fp32->fp16 into DRAM (SWDGE cast DMA), then loaded
     transposed into SBUF via the xbar DMA-transpose path -> inT [h, s].
  2. WH^T tiles [h'=128, s=512] = W_a-tile^T @ inT  (PE, fp16).
  3. tanh(WH^T + U_a h_t) fused on ACT (per-partition bias), fp16 out.
  4. score = v_a^T tanh(...) via M=1 matmuls accumulated over h'-tiles.
  5. softmax (minus-max) + gaussian window + 1/sum on DVE/ACT rows.
  6. weights row replicated to 128 partitions via ones-matmul; context^T
     computed on DVE with fused multiply+reduce against inT.
  7. final tanh([ctx, h_t] @ W_att) via fp16 matmuls, fp32 out.
"""

import math
from contextlib import ExitStack

import numpy as np

B, S, H, SIZE = 32, 2048, 1024, 1024
N_CORES = 8
BPC = B // N_CORES
P = 128
NB = 512

_compiled = None


def _build(bpc=BPC, s=S, h=H, size=SIZE, debug=False):
    import concourse.bacc as bacc
    import concourse.mybir as mybir
    import concourse.tile as tile

    F32 = mybir.dt.float32
    F16 = mybir.dt.float16
    AF = mybir.ActivationFunctionType
    ALU = mybir.AluOpType
    AX = mybir.AxisListType

    KT = h // P          # k-tiles over H
    SQ = s // NB         # s blocks of 512
    KT2 = 2 * h // P     # k-tiles over 2H (final projection)
    NO = size // NB      # output blocks
    H2 = h // 2
    denom = 2.0 * ((s // 2) / 2.0) ** 2
    inv_sq_denom = 1.0 / math.sqrt(denom)

    nc = bacc.Bacc("TRN2", target_bir_lowering=False, debug=debug)

    x = nc.dram_tensor("inputs", [bpc, s, h], F32, kind="ExternalInput").ap()
    W_p = nc.dram_tensor("W_p", [h, h], F32, kind="ExternalInput").ap()
    v_p = nc.dram_tensor("v_p", [h, 1], F32, kind="ExternalInput").ap()
    W_a = nc.dram_tensor("W_a", [h, h], F32, kind="ExternalInput").ap()
    U_a = nc.dram_tensor("U_a", [h, h], F32, kind="ExternalInput").ap()
    v_a = nc.dram_tensor("v_a", [h, 1], F32, kind="ExternalInput").ap()
    W_att = nc.dram_tensor("W_att", [2 * h, size], F32, kind="ExternalInput").ap()
    out = nc.dram_tensor("out", [bpc, size], F32, kind="ExternalOutput").ap()

    with tile.TileContext(nc) as tc, ExitStack() as ctx:
        dp = ctx.enter_context(tc.tile_pool(name="dram", bufs=2, space="DRAM"))
        sb = ctx.enter_context(tc.tile_pool(name="sb", bufs=1))
        ps = ctx.enter_context(tc.tile_pool(name="ps", bufs=1, space="PSUM"))

        inT_tiles = [None] * bpc

        def emit_input_dma(b):
            # single fully-contiguous fp32->fp16 cast DMA (SWDGE), then xbar
            # transposes of the k-th 128-column slice.
            xf = dp.tile([s, h], F16, name=f"xf16_{b}", tag="xf16")
            nc.gpsimd.dma_start(xf[:], x[b])
            inT = sb.tile([P, KT, s], F16, name=f"inT_{b}", tag="big", bufs=2)
            for k in range(KT):
                nc.sync.dma_start(inT[:, k, :], xf[:, k * P:(k + 1) * P],
                                  transpose=True)
            inT_tiles[b] = inT

        # ---- SWDGE queue order: b0 cast, U_a, W_a, v_a, b1 cast, W_p,
        # then b2/b3/W_att from the batch loop. HWDGE carries ONLY the xbar
        # transposes (mixing copy-DMAs with transposes serializes globally),
        # plus a few tiny DMAs emitted before the first transpose / at exit.
        def load_weight_f16(name, dram_ap, kt, n, tag, bufs):
            w16 = sb.tile([P, kt, n], F16, name=name, tag=tag, bufs=bufs)
            nc.gpsimd.dma_start(w16[:],
                                dram_ap.rearrange("(k p) n -> p k n", p=P))
            return w16

        htb = sb.tile([bpc, h], F32, name="htb", tag="htb")
        nc.scalar.dma_start(htb[:], x[:, s - 1, :])
        vp_rep = sb.tile([bpc, h], F32, name="vp_rep", tag="vp_rep")
        for i in range(bpc):
            nc.scalar.dma_start(vp_rep[i:i + 1, :], v_p.rearrange("n o -> o n"))
        emit_input_dma(0)
        ua_sb = load_weight_f16("ua_sb", U_a, KT, h, "ua", 1)
        wa_sb = load_weight_f16("wa_sb", W_a, KT, h, "wa", 1)
        va_sb = sb.tile([P, KT], F16, name="va_sb", tag="va")
        nc.gpsimd.dma_start(va_sb[:], v_a.rearrange("(k p) o -> p (k o)", p=P))
        emit_input_dma(1)
        wp_sb = load_weight_f16("wp_sb", W_p, KT, h, "wend", 1)

        # ---- constants ----
        ident_io = sb.tile([bpc, bpc], F32, name="ident_io", tag="ident_io")
        nc.gpsimd.iota(ident_io[:], pattern=[[1, bpc]], base=0,
                       channel_multiplier=-1,
                       allow_small_or_imprecise_dtypes=True)
        ident = sb.tile([bpc, bpc], F32, name="ident", tag="ident")
        nc.vector.tensor_scalar(ident[:], ident_io[:], 0.0, None,
                                op0=ALU.is_equal)
        ones1 = sb.tile([1, P], F16, name="ones1", tag="ones1")
        nc.vector.memset(ones1[:], 1.0)
        pos_row = sb.tile([1, s], F16, name="pos_row", tag="pos")
        nc.gpsimd.iota(pos_row[:], pattern=[[1, s]], base=0,
                       channel_multiplier=0,
                       allow_small_or_imprecise_dtypes=True)

        # ---- h_t extraction + transposes ----


        htT = sb.tile([P, KT, bpc], F32, name="htT", tag="htT")
        htT16 = sb.tile([P, KT, bpc], F16, name="htT16", tag="htT16")
        combT = sb.tile([P, KT2, bpc], F16, name="combT", tag="combT")
        for k in range(KT):
            pt = ps.tile([P, bpc], F32, name=f"pt_{k}", tag="wh", bufs=4)
            nc.tensor.transpose(pt[:], htb[:, k * P:(k + 1) * P], ident[:])
            nc.scalar.activation(htT[:, k, :], pt[:], AF.Copy)
            nc.vector.tensor_copy(htT16[:, k, :], pt[:])
            nc.vector.tensor_copy(combT[:, KT + k, :], pt[:])

        # ---- WT = h_t @ U_a, then transpose -> wtT ----
        wt_row = sb.tile([bpc, h], F32, name="wt_row", tag="wt_row")
        for n2 in range(h // NB):
            pwt = ps.tile([bpc, NB], F32, name=f"pwt_{n2}", tag="sc", bufs=4)
            for k in range(KT):
                nc.tensor.matmul(pwt[:], htT16[:, k, :],
                                 ua_sb[:, k, n2 * NB:(n2 + 1) * NB],
                                 start=(k == 0), stop=(k == KT - 1))
            nc.scalar.activation(wt_row[:, n2 * NB:(n2 + 1) * NB], pwt[:], AF.Copy)
        wtT = sb.tile([P, KT, bpc], F32, name="wtT", tag="wtT")
        for k in range(KT):
            pt2 = ps.tile([P, bpc], F32, name=f"pt2_{k}", tag="wh", bufs=4)
            nc.tensor.transpose(pt2[:], wt_row[:, k * P:(k + 1) * P], ident[:])
            nc.scalar.activation(wtT[:, k, :], pt2[:], AF.Copy)

        # ---- p_t = sigmoid(tanh(h_t @ W_p) @ v_p) * s ----
        tanhP = sb.tile([bpc, h], F32, name="tanhP", tag="tanhP")
        for n2 in range(h // NB):
            pwp = ps.tile([bpc, NB], F32, name=f"pwp_{n2}", tag="sc", bufs=4)
            for k in range(KT):
                nc.tensor.matmul(pwp[:], htT16[:, k, :],
                                 wp_sb[:, k, n2 * NB:(n2 + 1) * NB],
                                 start=(k == 0), stop=(k == KT - 1))
            nc.scalar.activation(tanhP[:, n2 * NB:(n2 + 1) * NB], pwp[:], AF.Tanh)
        z2t = sb.tile([bpc, 1], F32, name="z2t", tag="z2t")
        nc.vector.scalar_tensor_tensor(
            tanhP[:], tanhP[:], 1.0, vp_rep[:],
            op0=ALU.mult, op1=ALU.mult, accum_out=z2t[:])
        pz = ps.tile([1, bpc], F32, name="pz", tag="sc", bufs=4)
        nc.tensor.transpose(pz[:], z2t[:], ident[:])
        sg_row = sb.tile([1, bpc], F32, name="sg_row", tag="sg_row")
        nc.scalar.activation(sg_row[:], pz[:], AF.Sigmoid)
        p_row = sb.tile([1, bpc], F32, name="p_row", tag="p_row")
        nc.vector.tensor_scalar_mul(p_row[:], sg_row[:], float(s))

        # ---- watt (emitted during batch 1 prefetch; declared here) ----
        watt_holder = [None]

        def emit_watt_dma():
            watt_holder[0] = load_weight_f16("watt_sb", W_att, KT2, size,
                                             "wend", 1)

        # ---- main batch loop ----
        for b in range(bpc):
            if b + 2 < bpc:
                emit_input_dma(b + 2)
            if b == 1 or bpc <= 2:
                emit_watt_dma()
            inT = inT_tiles[b]

            sc_ps = [ps.tile([1, NB], F32, name=f"sc_{b}_{q}", tag="sc", bufs=4)
                     for q in range(SQ)]

            def emit_va_mms(hp, tanh_tiles):
                for q in range(SQ):
                    nc.tensor.matmul(sc_ps[q][:], va_sb[:, hp:hp + 1],
                                     tanh_tiles[q][:],
                                     start=(hp == 0), stop=(hp == KT - 1),
                                     skip_group_check=True)

            # v_a matmuls run one hp-group behind the main matmuls so the PE
            # never waits on ACT's tanh.
            pend = None
            for hp in range(KT):
                wh_ps = [ps.tile([P, NB], F32, name=f"wh_{b}_{hp}_{q}",
                                 tag="wh", bufs=4) for q in range(SQ)]
                for k in range(KT):
                    lhsT = wa_sb[:, k, hp * P:(hp + 1) * P]
                    for q in range(SQ):
                        nc.tensor.matmul(
                            wh_ps[q][:], lhsT, inT[:, k, q * NB:(q + 1) * NB],
                            start=(k == 0), stop=(k == KT - 1),
                            skip_group_check=True)
                if pend is not None:
                    emit_va_mms(hp - 1, pend)
                ths = []
                for q in range(SQ):
                    th = sb.tile([P, NB], F16, name=f"th_{b}_{hp}_{q}",
                                 tag="tanh", bufs=8)
                    nc.scalar.activation(th[:], wh_ps[q][:], AF.Tanh,
                                         bias=wtT[:, hp, b:b + 1])
                    ths.append(th)
                pend = ths
            emit_va_mms(KT - 1, pend)

            # ---- softmax * gaussian (rows on partition 0) ----
            score = sb.tile([1, s], F32, name=f"score_{b}", tag="score")
            for q in range(SQ):
                nc.vector.tensor_copy(score[0:1, q * NB:(q + 1) * NB],
                                      sc_ps[q][:])
            nmx = sb.tile([1, 1], F32, name=f"nmx_{b}", tag="nmx", bufs=2)
            nc.vector.tensor_reduce(nmx[:], score[:], axis=AX.X, op=ALU.max,
                                    negate=True)
            e1 = sb.tile([1, s], F16, name=f"e1_{b}", tag="e1")
            nc.scalar.activation(e1[:], score[:], AF.Exp, bias=nmx[0:1, 0:1])
            se = sb.tile([1, 1], F32, name=f"se_{b}", tag="se", bufs=2)
            nc.vector.tensor_reduce(se[:], e1[:], axis=AX.X, op=ALU.add)
            rr = sb.tile([1, 1], F32, name=f"rr_{b}", tag="rr", bufs=2)
            nc.vector.reciprocal(rr[:], se[:])
            dr = sb.tile([1, s], F16, name=f"dr_{b}", tag="gA")
            nc.vector.tensor_scalar(dr[:], pos_row[:], p_row[0:1, b:b + 1],
                                    inv_sq_denom, op0=ALU.subtract,
                                    op1=ALU.mult)
            d2 = sb.tile([1, s], F16, name=f"d2_{b}", tag="gB")
            nc.vector.tensor_mul(d2[:], dr[:], dr[:])
            gr = sb.tile([1, s], F16, name=f"gr_{b}", tag="gA")
            nc.scalar.activation(gr[:], d2[:], AF.Exp, scale=-1.0)
            wu = sb.tile([1, s], F16, name=f"wu_{b}", tag="gB")
            nc.vector.scalar_tensor_tensor(wu[:], e1[:], rr[0:1, 0:1], gr[:],
                                           op0=ALU.mult, op1=ALU.mult)

            # ---- replicate weights row across partitions ----
            wrep = sb.tile([P, s], F16, name=f"wrep_{b}", tag="wrep", bufs=2)
            for q in range(SQ):
                pwr = ps.tile([P, NB], F32, name=f"pwr_{b}_{q}", tag="sc",
                              bufs=4)
                nc.tensor.matmul(pwr[:], ones1[0:1, :],
                                 wu[0:1, q * NB:(q + 1) * NB],
                                 start=True, stop=True, skip_group_check=True)
                nc.scalar.activation(wrep[:, q * NB:(q + 1) * NB], pwr[:],
                                     AF.Copy)

            # ---- context^T via fused multiply+reduce on DVE ----
            ctxa = sb.tile([P, KT], F32, name=f"ctxa_{b}", tag="ctxa", bufs=2)
            for k in range(KT):
                nc.vector.scalar_tensor_tensor(
                    inT[:, k, :], inT[:, k, :], 1.0, wrep[:],
                    op0=ALU.mult, op1=ALU.mult,
                    accum_out=ctxa[:, k:k + 1])
                nc.vector.tensor_copy(combT[:, k, b:b + 1], ctxa[:, k:k + 1])

        # ---- final projection: tanh([ctx, h_t] @ W_att) ----
        watt_sb = watt_holder[0]
        outsb = sb.tile([bpc, size], F32, name="outsb", tag="outsb")
        pfs = [ps.tile([bpc, NB], F32, name=f"pf_{n2}", tag="sc", bufs=4)
               for n2 in range(NO)]
        for kk in list(range(KT, KT2)) + list(range(KT)):
            for n2 in range(NO):
                nc.tensor.matmul(pfs[n2][:], combT[:, kk, :],
                                 watt_sb[:, kk, n2 * NB:(n2 + 1) * NB],
                                 start=(kk == KT), stop=(kk == KT - 1),
                                 skip_group_check=True)
        for n2 in range(NO):
            nc.scalar.activation(outsb[:, n2 * NB:(n2 + 1) * NB], pfs[n2][:],
                                 AF.Tanh)
        nc.scalar.dma_start(out[:], outsb[:])

    nc.compile()
    return nc


def kernel(**inputs):
    global _compiled
    from concourse import bass_utils

    if _compiled is None:
        _compiled = _build()

    x = np.ascontiguousarray(np.asarray(inputs["inputs"], dtype=np.float32))
    weights = {
        k: np.ascontiguousarray(np.asarray(inputs[k], dtype=np.float32))
        for k in ("W_p", "v_p", "W_a", "U_a", "v_a", "W_att")
    }
    in_maps = [
        {"inputs": x[i * BPC:(i + 1) * BPC], **weights} for i in range(N_CORES)
    ]
    res = bass_utils.run_bass_kernel_spmd(_compiled, in_maps,
                                          list(range(N_CORES)))
    return np.concatenate([res.results[i]["out"] for i in range(N_CORES)],
                          axis=0).astype(np.float32)



# revision 2
# speedup vs baseline: 1.6620x; 1.6620x over previous
"""Trainium2 Bass kernel: Bahdanau local-p attention (B=32, S=2048, H=1024).

Sharding: data-parallel over batch. Each of the 8 cores processes B/8 = 4
batches end-to-end (weights replicated); no collectives.

Host-side prep (inside kernel(), numpy): inputs are transposed to [H, S],
cast to fp8-e4m3 and packed into the exact SBUF layout (partition-major,
DoubleRow k-pair interleave), so the device does single contiguous DMAs —
no cast DMAs, no DMA transposes. Weights are likewise pre-cast (fp8 for
W_a, fp16 for the rest) and packed partition-major.

Per-core dataflow:
  1. inT [128, K8, 2, S] fp8 loaded per batch (contiguous DMA).
  2. WH^T tiles [128, 512] = W_a-tile^T @ inT on PE in fp8 with
     perf_mode=DoubleRow (2 MACs/cell/cycle, K=256 per call).
  3. tanh(WH^T + U_a h_t) fused on ACT (per-partition bias), fp16 out.
  4. score = v_a^T tanh(...) via M=1 fp16 matmuls accumulated over hp.
  5. softmax (minus-max) + gaussian window + 1/sum on DVE/ACT rows.
  6. weights row replicated to 128 partitions via ones-matmul; context^T
     computed on DVE with fused multiply+reduce against fp8 inT.
  7. final tanh([ctx, h_t] @ W_att) via fp16 matmuls, fp32 out.
"""

import math
from contextlib import ExitStack

import numpy as np

B, S, H, SIZE = 32, 2048, 1024, 1024
N_CORES = 8
BPC = B // N_CORES
P = 128
NB = 512
KT = H // P        # 8  k-tiles over H
K8 = H // (2 * P)  # 4  double-k-tiles (DoubleRow)
SQ = S // NB       # 4  s blocks
KT2 = 2 * H // P   # 16 k-tiles over 2H (final projection)
NO = SIZE // NB    # 2  output blocks

_compiled = None


def _build(bpc=BPC, s=S, h=H, size=SIZE, debug=False):
    import concourse.bacc as bacc
    import concourse.mybir as mybir
    import concourse.tile as tile

    F32 = mybir.dt.float32
    F16 = mybir.dt.float16
    F8 = mybir.dt.float8e4
    AF = mybir.ActivationFunctionType
    ALU = mybir.AluOpType
    AX = mybir.AxisListType
    DR = mybir.MatmulPerfMode.DoubleRow

    denom = 2.0 * ((s // 2) / 2.0) ** 2
    inv_sq_denom = 1.0 / math.sqrt(denom)

    nc = bacc.Bacc("TRN2", target_bir_lowering=False, debug=debug)

    x8 = nc.dram_tensor("x8", [bpc, P, K8, 2, s], F8, kind="ExternalInput").ap()
    ht = nc.dram_tensor("ht", [bpc, h], F32, kind="ExternalInput").ap()
    vp = nc.dram_tensor("vp", [bpc, h], F32, kind="ExternalInput").ap()
    wa8 = nc.dram_tensor("wa8", [P, K8, 2, h], F8, kind="ExternalInput").ap()
    ua16 = nc.dram_tensor("ua16", [P, KT, h], F16, kind="ExternalInput").ap()
    wp16 = nc.dram_tensor("wp16", [P, KT, h], F16, kind="ExternalInput").ap()
    watt16 = nc.dram_tensor("watt16", [P, KT2, size], F16,
                            kind="ExternalInput").ap()
    va16 = nc.dram_tensor("va16", [P, KT], F16, kind="ExternalInput").ap()
    out = nc.dram_tensor("out", [bpc, size], F32, kind="ExternalOutput").ap()

    with tile.TileContext(nc) as tc, ExitStack() as ctx:
        sb = ctx.enter_context(tc.tile_pool(name="sb", bufs=1))
        ps = ctx.enter_context(tc.tile_pool(name="ps", bufs=1, space="PSUM"))

        inT_tiles = [None] * bpc

        def emit_input_dma(b):
            inT = sb.tile([P, K8, 2, s], F8, name=f"inT_{b}", tag="big",
                          bufs=3)
            nc.sync.dma_start(inT[:], x8[b])
            inT_tiles[b] = inT

        # ---- startup DMAs: smalls on scalar queue, weights on SWDGE,
        # inputs on HWDGE(sync). ua before wa: the wtT chain (first tanh
        # bias) only needs ua; main matmuls need wa slightly later.
        htb = sb.tile([bpc, h], F32, name="htb", tag="htb")
        nc.scalar.dma_start(htb[:], ht[:])
        vp_rep = sb.tile([bpc, h], F32, name="vp_rep", tag="vp_rep")
        nc.scalar.dma_start(vp_rep[:], vp[:])
        emit_input_dma(0)
        ua_sb = sb.tile([P, KT, h], F16, name="ua_sb", tag="ua")
        nc.gpsimd.dma_start(ua_sb[:], ua16[:])
        wa_sb = sb.tile([P, K8, 2, h], F8, name="wa_sb", tag="wa")
        nc.gpsimd.dma_start(wa_sb[:], wa8[:])
        va_sb = sb.tile([P, KT], F16, name="va_sb", tag="va")
        nc.gpsimd.dma_start(va_sb[:], va16[:])
        emit_input_dma(1)
        wp_sb = sb.tile([P, KT, h], F16, name="wp_sb", tag="wend")
        nc.gpsimd.dma_start(wp_sb[:], wp16[:])

        # ---- constants ----
        ident_io = sb.tile([bpc, bpc], F32, name="ident_io", tag="ident_io")
        nc.gpsimd.iota(ident_io[:], pattern=[[1, bpc]], base=0,
                       channel_multiplier=-1,
                       allow_small_or_imprecise_dtypes=True)
        ident = sb.tile([bpc, bpc], F32, name="ident", tag="ident")
        nc.vector.tensor_scalar(ident[:], ident_io[:], 0.0, None,
                                op0=ALU.is_equal)
        ones1 = sb.tile([1, P], F16, name="ones1", tag="ones1")
        nc.vector.memset(ones1[:], 1.0)
        pos_row = sb.tile([1, s], F16, name="pos_row", tag="pos")
        nc.gpsimd.iota(pos_row[:], pattern=[[1, s]], base=0,
                       channel_multiplier=0,
                       allow_small_or_imprecise_dtypes=True)

        # ---- h_t transposes ----
        htT16 = sb.tile([P, KT, bpc], F16, name="htT16", tag="htT16")
        combT = sb.tile([P, KT2, bpc], F16, name="combT", tag="combT")
        for k in range(KT):
            pt = ps.tile([P, bpc], F32, name=f"pt_{k}", tag="wh", bufs=4)
            nc.tensor.transpose(pt[:], htb[:, k * P:(k + 1) * P], ident[:])
            nc.vector.tensor_copy(htT16[:, k, :], pt[:])
            nc.vector.tensor_copy(combT[:, KT + k, :], pt[:])

        # ---- WT = h_t @ U_a, then transpose -> wtT ----
        wt_row = sb.tile([bpc, h], F32, name="wt_row", tag="wt_row")
        for n2 in range(h // NB):
            pwt = ps.tile([bpc, NB], F32, name=f"pwt_{n2}", tag="sc", bufs=4)
            for k in range(KT):
                nc.tensor.matmul(pwt[:], htT16[:, k, :],
                                 ua_sb[:, k, n2 * NB:(n2 + 1) * NB],
                                 start=(k == 0), stop=(k == KT - 1))
            nc.scalar.activation(wt_row[:, n2 * NB:(n2 + 1) * NB], pwt[:],
                                 AF.Copy)
        wtT = sb.tile([P, KT, bpc], F32, name="wtT", tag="wtT")
        for k in range(KT):
            pt2 = ps.tile([P, bpc], F32, name=f"pt2_{k}", tag="wh", bufs=4)
            nc.tensor.transpose(pt2[:], wt_row[:, k * P:(k + 1) * P], ident[:])
            nc.scalar.activation(wtT[:, k, :], pt2[:], AF.Copy)

        # ---- p_t = sigmoid(tanh(h_t @ W_p) @ v_p) * s  (emitted inside
        # batch 0 after the main matmuls so it doesn't block the PE queue
        # on the wp DMA; only needed by batch 0's gaussian) ----
        p_row = sb.tile([1, bpc], F32, name="p_row", tag="p_row")

        def emit_pt():
            tanhP = sb.tile([bpc, h], F32, name="tanhP", tag="tanhP")
            for n2 in range(h // NB):
                pwp = ps.tile([bpc, NB], F32, name=f"pwp_{n2}", tag="sc",
                              bufs=4)
                for k in range(KT):
                    nc.tensor.matmul(pwp[:], htT16[:, k, :],
                                     wp_sb[:, k, n2 * NB:(n2 + 1) * NB],
                                     start=(k == 0), stop=(k == KT - 1))
                nc.scalar.activation(tanhP[:, n2 * NB:(n2 + 1) * NB], pwp[:],
                                     AF.Tanh)
            z2t = sb.tile([bpc, 1], F32, name="z2t", tag="z2t")
            nc.vector.scalar_tensor_tensor(
                tanhP[:], tanhP[:], 1.0, vp_rep[:],
                op0=ALU.mult, op1=ALU.mult, accum_out=z2t[:])
            pz = ps.tile([1, bpc], F32, name="pz", tag="sc", bufs=4)
            nc.tensor.transpose(pz[:], z2t[:], ident[:])
            sg_row = sb.tile([1, bpc], F32, name="sg_row", tag="sg_row")
            nc.scalar.activation(sg_row[:], pz[:], AF.Sigmoid)
            nc.vector.tensor_scalar_mul(p_row[:], sg_row[:], float(s))

        # ---- watt (emitted during batch 1 prefetch; declared here) ----
        watt_holder = [None]

        def emit_watt_dma():
            w16 = sb.tile([P, KT2, size], F16, name="watt_sb", tag="wend")
            nc.gpsimd.dma_start(w16[:], watt16[:])
            watt_holder[0] = w16

        # ---- main batch loop ----
        for b in range(bpc):
            if b + 2 < bpc:
                emit_input_dma(b + 2)
            if b == 1 or bpc <= 2:
                emit_watt_dma()
            inT = inT_tiles[b]

            sc_ps = [ps.tile([1, NB], F32, name=f"sc_{b}_{q}", tag="sc",
                             bufs=4) for q in range(SQ)]

            def emit_va_mms(hp, tanh_tiles):
                for q in range(SQ):
                    nc.tensor.matmul(sc_ps[q][:], va_sb[:, hp:hp + 1],
                                     tanh_tiles[q][:],
                                     start=(hp == 0), stop=(hp == KT - 1),
                                     skip_group_check=True)

            # v_a matmuls run one hp-group behind the main matmuls so the
            # PE never waits on ACT's tanh.
            pend = None
            for hp in range(KT):
                wh_ps = [ps.tile([P, NB], F32, name=f"wh_{b}_{hp}_{q}",
                                 tag="wh", bufs=4) for q in range(SQ)]
                for k2 in range(K8):
                    lhsT = wa_sb[:, k2, :, hp * P:(hp + 1) * P]
                    for q in range(SQ):
                        nc.tensor.matmul(
                            wh_ps[q][:], lhsT,
                            inT[:, k2, :, q * NB:(q + 1) * NB],
                            start=(k2 == 0), stop=(k2 == K8 - 1),
                            perf_mode=DR, skip_group_check=True)
                if pend is not None:
                    emit_va_mms(hp - 1, pend)
                ths = []
                for q in range(SQ):
                    th = sb.tile([P, NB], F16, name=f"th_{b}_{hp}_{q}",
                                 tag="tanh", bufs=8)
                    nc.scalar.activation(th[:], wh_ps[q][:], AF.Tanh,
                                         bias=wtT[:, hp, b:b + 1])
                    ths.append(th)
                pend = ths
            emit_va_mms(KT - 1, pend)
            if b == 0:
                emit_pt()

            # ---- softmax * gaussian (rows on partition 0) ----
            score = sb.tile([1, s], F32, name=f"score_{b}", tag="score")
            for q in range(SQ):
                nc.vector.tensor_copy(score[0:1, q * NB:(q + 1) * NB],
                                      sc_ps[q][:])
            nmx = sb.tile([1, 1], F32, name=f"nmx_{b}", tag="nmx", bufs=2)
            nc.vector.tensor_reduce(nmx[:], score[:], axis=AX.X, op=ALU.max,
                                    negate=True)
            e1 = sb.tile([1, s], F16, name=f"e1_{b}", tag="e1")
            nc.scalar.activation(e1[:], score[:], AF.Exp, bias=nmx[0:1, 0:1])
            se = sb.tile([1, 1], F32, name=f"se_{b}", tag="se", bufs=2)
            nc.vector.tensor_reduce(se[:], e1[:], axis=AX.X, op=ALU.add)
            rr = sb.tile([1, 1], F32, name=f"rr_{b}", tag="rr", bufs=2)
            nc.vector.reciprocal(rr[:], se[:])
            dr = sb.tile([1, s], F16, name=f"dr_{b}", tag="gA")
            nc.vector.tensor_scalar(dr[:], pos_row[:], p_row[0:1, b:b + 1],
                                    inv_sq_denom, op0=ALU.subtract,
                                    op1=ALU.mult)
            d2 = sb.tile([1, s], F16, name=f"d2_{b}", tag="gB")
            nc.vector.tensor_mul(d2[:], dr[:], dr[:])
            gr = sb.tile([1, s], F16, name=f"gr_{b}", tag="gA")
            nc.scalar.activation(gr[:], d2[:], AF.Exp, scale=-1.0)
            wu = sb.tile([1, s], F16, name=f"wu_{b}", tag="gB")
            nc.vector.scalar_tensor_tensor(wu[:], e1[:], rr[0:1, 0:1], gr[:],
                                           op0=ALU.mult, op1=ALU.mult)

            # ---- replicate weights row across partitions ----
            wrep = sb.tile([P, s], F16, name=f"wrep_{b}", tag="wrep", bufs=2)
            for q in range(SQ):
                pwr = ps.tile([P, NB], F32, name=f"pwr_{b}_{q}", tag="sc",
                              bufs=4)
                nc.tensor.matmul(pwr[:], ones1[0:1, :],
                                 wu[0:1, q * NB:(q + 1) * NB],
                                 start=True, stop=True, skip_group_check=True)
                nc.scalar.activation(wrep[:, q * NB:(q + 1) * NB], pwr[:],
                                     AF.Copy)

            # ---- context^T via fused multiply+reduce on DVE ----
            ctxa = sb.tile([P, KT], F32, name=f"ctxa_{b}", tag="ctxa", bufs=2)
            for k2 in range(K8):
                for i in range(2):
                    kk = 2 * k2 + i
                    nc.vector.scalar_tensor_tensor(
                        inT[:, k2, i, :], inT[:, k2, i, :], 1.0, wrep[:],
                        op0=ALU.mult, op1=ALU.mult,
                        accum_out=ctxa[:, kk:kk + 1])
                    nc.vector.tensor_copy(combT[:, kk, b:b + 1],
                                          ctxa[:, kk:kk + 1])

        # ---- final projection: tanh([ctx, h_t] @ W_att) ----
        watt_sb = watt_holder[0]
        outsb = sb.tile([bpc, size], F32, name="outsb", tag="outsb")
        pfs = [ps.tile([bpc, NB], F32, name=f"pf_{n2}", tag="sc", bufs=4)
               for n2 in range(NO)]
        for kk in list(range(KT, KT2)) + list(range(KT)):
            for n2 in range(NO):
                nc.tensor.matmul(pfs[n2][:], combT[:, kk, :],
                                 watt_sb[:, kk, n2 * NB:(n2 + 1) * NB],
                                 start=(kk == KT), stop=(kk == KT - 1),
                                 skip_group_check=True)
        for n2 in range(NO):
            nc.scalar.activation(outsb[:, n2 * NB:(n2 + 1) * NB], pfs[n2][:],
                                 AF.Tanh)
        nc.scalar.dma_start(out[:], outsb[:])

    nc.compile()
    return nc


def build_in_maps(inputs):
    """Host-side packing: shard batch over cores, transpose/cast/pack
    inputs and weights into the exact device layouts."""
    import ml_dtypes

    F8 = ml_dtypes.float8_e4m3
    F16 = np.float16

    x = np.asarray(inputs["inputs"], dtype=np.float32)
    # [B, S, H] -> fp8 -> [B, P, K8, 2, S] (partition-major, k-pair pairs)
    x8 = np.ascontiguousarray(
        x.astype(F8).transpose(0, 2, 1)
        .reshape(B, K8, 2, P, S).transpose(0, 3, 1, 2, 4))
    htf = np.ascontiguousarray(x[:, -1, :])

    W_a = np.asarray(inputs["W_a"], dtype=np.float32)
    wa8 = np.ascontiguousarray(
        W_a.astype(F8).reshape(K8, 2, P, H).transpose(2, 0, 1, 3))
    ua16 = np.ascontiguousarray(
        np.asarray(inputs["U_a"], np.float32).astype(F16)
        .reshape(KT, P, H).transpose(1, 0, 2))
    wp16 = np.ascontiguousarray(
        np.asarray(inputs["W_p"], np.float32).astype(F16)
        .reshape(KT, P, H).transpose(1, 0, 2))
    watt16 = np.ascontiguousarray(
        np.asarray(inputs["W_att"], np.float32).astype(F16)
        .reshape(KT2, P, SIZE).transpose(1, 0, 2))
    va16 = np.ascontiguousarray(
        np.asarray(inputs["v_a"], np.float32)[:, 0].astype(F16)
        .reshape(KT, P).T)
    vp32 = np.ascontiguousarray(
        np.broadcast_to(np.asarray(inputs["v_p"], np.float32)[:, 0],
                        (BPC, H)))

    shared = {"wa8": wa8, "ua16": ua16, "wp16": wp16, "watt16": watt16,
              "va16": va16, "vp": vp32}
    return [
        {"x8": x8[i * BPC:(i + 1) * BPC],
         "ht": htf[i * BPC:(i + 1) * BPC], **shared}
        for i in range(N_CORES)
    ]


def kernel(**inputs):
    global _compiled
    from concourse import bass_utils

    if _compiled is None:
        _compiled = _build()

    in_maps = build_in_maps(inputs)
    res = bass_utils.run_bass_kernel_spmd(_compiled, in_maps,
                                          list(range(N_CORES)))
    return np.concatenate([res.results[i]["out"] for i in range(N_CORES)],
                          axis=0).astype(np.float32)


# revision 10
# speedup vs baseline: 1.9163x; 1.1530x over previous
"""Trainium2 Bass kernel: Bahdanau local-p attention (B=32, S=2048, H=1024).

Sharding: data-parallel over batch. Each of the 8 cores processes B/8 = 4
batches end-to-end (weights replicated); no collectives.

Host-side prep (inside kernel(), numpy): inputs are transposed to [H, S],
cast to fp8-e4m3 and packed into the exact SBUF layout (partition-major,
DoubleRow k-pair interleave) so the device does single contiguous DMAs —
no cast DMAs, no DMA transposes. The tiny h_t-only projections (p_t,
U_a h_t bias, the h_t half of the concat) are precomputed on host in f32
(0.2% of FLOPs); all S-dimension work runs on device.

Per-core dataflow:
  1. inT [128, K8, 2, S] fp8 loaded per batch (contiguous DMA, batch 0
     split into k2 chunks so the PE can start on the first 512 KB).
  2. WH^T tiles [128, 512] = W_a-tile^T @ inT on PE in fp8 with
     perf_mode=DoubleRow (2 MACs/cell/cycle, K=256 per call).
  3. tanh(WH^T + (U_a h_t)) fused on ACT (per-partition bias), fp8 out,
     hp-pairs interleaved for the DoubleRow v_a dot.
  4. score = v_a^T tanh(...) via M=1 fp8 DoubleRow matmuls.
  5. softmax (minus-max) + gaussian window + 1/sum on DVE/ACT rows.
  6. weights row replicated to 128 partitions via ones-matmul; context^T
     via fused multiply+reduce split across DVE (even k) / GpSimd (odd k).
     wrep + ctx for batch b are emitted after batch b+1's first two
     hp-groups so the in-order PE queue never stalls on the softmax chain.
  7. final tanh([ctx, h_t] @ W_att) fp16 matmuls interleaved with the
     tail context slices, fp32 out.
"""

import math
from contextlib import ExitStack

import numpy as np

B, S, H, SIZE = 32, 2048, 1024, 1024
N_CORES = 8
BPC = B // N_CORES
P = 128
NB = 512
KT = H // P        # 8  k-tiles over H
K8 = H // (2 * P)  # 4  double-k-tiles (DoubleRow)
K8V = KT // 2      # 4  hp-pairs for the v_a dot
SQ = S // NB       # 4  s blocks
KT2 = 2 * H // P   # 16 k-tiles over 2H (final projection)
NO = SIZE // NB    # 2  output blocks

_compiled = None


def _build(bpc=BPC, s=S, h=H, size=SIZE, debug=False):
    import concourse.bacc as bacc
    import concourse.mybir as mybir
    import concourse.tile as tile

    F32 = mybir.dt.float32
    F16 = mybir.dt.float16
    F8 = mybir.dt.float8e4
    AF = mybir.ActivationFunctionType
    ALU = mybir.AluOpType
    AX = mybir.AxisListType
    DR = mybir.MatmulPerfMode.DoubleRow

    denom = 2.0 * ((s // 2) / 2.0) ** 2
    inv_sq_denom = 1.0 / math.sqrt(denom)

    nc = bacc.Bacc("TRN2", target_bir_lowering=False, debug=debug)

    x8 = nc.dram_tensor("x8", [bpc, P, K8, 2, s], F8, kind="ExternalInput").ap()
    wa8 = nc.dram_tensor("wa8", [P, K8, 2, h], F8, kind="ExternalInput").ap()
    va8 = nc.dram_tensor("va8", [P, K8V, 2, 16], F8, kind="ExternalInput").ap()
    wtT_d = nc.dram_tensor("wtT", [P, KT, bpc], F32, kind="ExternalInput").ap()
    comb0 = nc.dram_tensor("comb0", [P, KT2, bpc], F16,
                           kind="ExternalInput").ap()
    prow_d = nc.dram_tensor("prow", [1, bpc], F32, kind="ExternalInput").ap()
    watt16 = nc.dram_tensor("watt16", [P, KT2, size], F16,
                            kind="ExternalInput").ap()
    out = nc.dram_tensor("out", [bpc, size], F32, kind="ExternalOutput").ap()

    with tile.TileContext(nc) as tc, ExitStack() as ctx:
        sb = ctx.enter_context(tc.tile_pool(name="sb", bufs=1))
        ps = ctx.enter_context(tc.tile_pool(name="ps", bufs=1, space="PSUM"))

        inT_tiles = [None] * bpc

        def emit_input_dma(b, chunked=False):
            inT = sb.tile([P, K8, 2, s], F8, name=f"inT_{b}", tag="big",
                          bufs=4)
            if chunked:
                for k2 in range(K8):
                    nc.sync.dma_start(inT[:, k2, :, :], x8[b, :, k2])
            else:
                nc.sync.dma_start(inT[:], x8[b])
            inT_tiles[b] = inT

        # ---- startup DMAs. Small host-precomputed tensors + weights on
        # the ACT HWDGE ring; the big inputs on the SP HWDGE ring.
        wtT = sb.tile([P, KT, bpc], F32, name="wtT", tag="wtT")
        nc.scalar.dma_start(wtT[:], wtT_d[:])
        combT = sb.tile([P, KT2, bpc], F16, name="combT", tag="combT")
        nc.scalar.dma_start(combT[:], comb0[:])
        p_row = sb.tile([1, bpc], F32, name="p_row", tag="p_row")
        nc.scalar.dma_start(p_row[:], prow_d[:])
        va_sb = sb.tile([P, K8V, 2, 16], F8, name="va_sb", tag="va")
        nc.scalar.dma_start(va_sb[:], va8[:])
        wa_sb = sb.tile([P, K8, 2, h], F8, name="wa_sb", tag="wa")
        nc.scalar.dma_start(wa_sb[:], wa8[:])
        emit_input_dma(0, chunked=True)
        emit_input_dma(1)
        emit_input_dma(2)
        emit_input_dma(3)

        # ---- constants ----
        ones1 = sb.tile([1, P], F16, name="ones1", tag="ones1")
        nc.vector.memset(ones1[:], 1.0)
        pos_row = sb.tile([1, s], F16, name="pos_row", tag="pos")
        nc.gpsimd.iota(pos_row[:], pattern=[[1, s]], base=0,
                       channel_multiplier=0,
                       allow_small_or_imprecise_dtypes=True)

        # ---- watt (emitted during batch 1; declared here) ----
        watt_holder = [None]

        def emit_watt_dma():
            w16 = sb.tile([P, KT2, size], F16, name="watt_sb", tag="wend")
            nc.scalar.dma_start(w16[:], watt16[:])
            watt_holder[0] = w16

        # deferred wrep+ctx emission for the previous batch
        pending = [None]

        # ---- main batch loop ----
        for b in range(bpc):
            if b == 1 or bpc <= 2:
                emit_watt_dma()
            inT = inT_tiles[b]

            # sc_ps allocated lazily at the first v_a matmul so the pool
            # rotation sequences them AFTER the deferred pwr tiles of the
            # previous batch (avoids a WAR cycle on the in-order queues).
            sc_ps = []

            def emit_va_mms(j, tanh_tiles):
                if j == 0:
                    sc_ps[:] = [ps.tile([16, NB], F32, name=f"sc_{b}_{q}",
                                        tag="sc", bufs=4) for q in range(SQ)]
                for q in range(SQ):
                    nc.tensor.matmul(sc_ps[q][:], va_sb[:, j],
                                     tanh_tiles[q][:],
                                     start=(j == 0), stop=(j == K8V - 1),
                                     perf_mode=DR, skip_group_check=True)

            # v_a matmuls run one hp-pair behind the main matmuls so the
            # PE never waits on ACT's tanh.
            pend2 = None
            for hp in range(KT):
                wh_ps = [ps.tile([P, NB], F32, name=f"wh_{b}_{hp}_{q}",
                                 tag="wh", bufs=4) for q in range(SQ)]
                for k2 in range(K8):
                    lhsT = wa_sb[:, k2, :, hp * P:(hp + 1) * P]
                    for q in range(SQ):
                        nc.tensor.matmul(
                            wh_ps[q][:], lhsT,
                            inT[:, k2, :, q * NB:(q + 1) * NB],
                            start=(k2 == 0), stop=(k2 == K8 - 1),
                            perf_mode=DR, skip_group_check=True)
                if hp == 2 and pending[0] is not None:
                    pending[0]()
                    pending[0] = None
                if hp % 2 == 0 and hp >= 2:
                    emit_va_mms(hp // 2 - 1, pend2)
                if hp % 2 == 0:
                    pend2 = [sb.tile([P, 2, NB], F8, name=f"th_{b}_{hp}_{q}",
                                     tag="tanh", bufs=8) for q in range(SQ)]
                for q in range(SQ):
                    nc.scalar.activation(pend2[q][:, hp % 2, :], wh_ps[q][:],
                                         AF.Tanh, bias=wtT[:, hp, b:b + 1])
            emit_va_mms(K8V - 1, pend2)

            # ---- softmax * gaussian (rows on partition 0) ----
            score = sb.tile([1, s], F32, name=f"score_{b}", tag="score")
            for q in range(SQ):
                nc.vector.tensor_copy(score[0:1, q * NB:(q + 1) * NB],
                                      sc_ps[q][0:1, :])
            nmx = sb.tile([1, 1], F32, name=f"nmx_{b}", tag="nmx", bufs=2)
            nc.vector.tensor_reduce(nmx[:], score[:], axis=AX.X, op=ALU.max,
                                    negate=True)
            e1 = sb.tile([1, s], F16, name=f"e1_{b}", tag="e1")
            nc.scalar.activation(e1[:], score[:], AF.Exp, bias=nmx[0:1, 0:1])
            se = sb.tile([1, 1], F32, name=f"se_{b}", tag="se", bufs=2)
            nc.vector.tensor_reduce(se[:], e1[:], axis=AX.X, op=ALU.add)
            rr = sb.tile([1, 1], F32, name=f"rr_{b}", tag="rr", bufs=2)
            nc.vector.reciprocal(rr[:], se[:])
            dr = sb.tile([1, s], F16, name=f"dr_{b}", tag="gA")
            nc.vector.tensor_scalar(dr[:], pos_row[:], p_row[0:1, b:b + 1],
                                    inv_sq_denom, op0=ALU.subtract,
                                    op1=ALU.mult)
            d2 = sb.tile([1, s], F16, name=f"d2_{b}", tag="gB")
            nc.vector.tensor_mul(d2[:], dr[:], dr[:])
            gr = sb.tile([1, s], F16, name=f"gr_{b}", tag="gA")
            nc.scalar.activation(gr[:], d2[:], AF.Exp, scale=-1.0)
            wu = sb.tile([1, s], F16, name=f"wu_{b}", tag="gB")
            nc.vector.scalar_tensor_tensor(wu[:], e1[:], rr[0:1, 0:1], gr[:],
                                           op0=ALU.mult, op1=ALU.mult)

            def make_wrep_ctx(b, inT, wu, final_cb=None):
                def emit():
                    # replicate weights row across partitions
                    wrep = sb.tile([P, s], F16, name=f"wrep_{b}", tag="wrep",
                                   bufs=2)
                    for q in range(SQ):
                        pwr = ps.tile([P, NB], F32, name=f"pwr_{b}_{q}",
                                      tag="sc", bufs=4)
                        nc.tensor.matmul(pwr[:], ones1[0:1, :],
                                         wu[0:1, q * NB:(q + 1) * NB],
                                         start=True, stop=True,
                                         skip_group_check=True)
                        nc.scalar.activation(wrep[:, q * NB:(q + 1) * NB],
                                             pwr[:], AF.Copy)
                    # context^T via fused multiply+reduce. DVE does 5
                    # slices with the fused STT (f32 internal accum);
                    # GpSimd takes 3 via mul into an f16 scratch (the fp8
                    # product would flush to zero) + reduce.
                    ctxa = sb.tile([P, KT], F32, name=f"ctxa_{b}",
                                   tag="ctxa", bufs=2)
                    for kk in range(KT):
                        k2, i = kk // 2, kk % 2
                        sl = inT[:, k2, i, :]
                        if kk in (1, 3, 5):
                            gscr = sb.tile([P, s], F16, name=f"gscr_{b}_{kk}",
                                           tag="gscr", bufs=2)
                            nc.gpsimd.tensor_mul(gscr[:], sl, wrep[:])
                            nc.vector.tensor_reduce(ctxa[:, kk:kk + 1],
                                                    gscr[:], axis=AX.X,
                                                    op=ALU.add)
                        else:
                            nc.vector.scalar_tensor_tensor(
                                sl, sl, 1.0, wrep[:],
                                op0=ALU.mult, op1=ALU.mult,
                                accum_out=ctxa[:, kk:kk + 1])
                        nc.vector.tensor_copy(combT[:, kk, b:b + 1],
                                              ctxa[:, kk:kk + 1])
                        if final_cb is not None:
                            final_cb(kk)
                return emit

            pending[0] = make_wrep_ctx(b, inT, wu)

        # ---- final projection: tanh([ctx, h_t] @ W_att) ----
        watt_sb = watt_holder[0]
        pfs = [ps.tile([bpc, NB], F32, name=f"pf_{n2}", tag="wh", bufs=4)
               for n2 in range(NO)]

        def emit_final(kk):
            for n2 in range(NO):
                nc.tensor.matmul(pfs[n2][:], combT[:, kk, :],
                                 watt_sb[:, kk, n2 * NB:(n2 + 1) * NB],
                                 start=(kk == KT), stop=(kk == KT - 1),
                                 skip_group_check=True)

        # h_t half runs during batch 3's softmax; the ctx half is
        # interleaved with batch 3's context slices.
        for kk in range(KT, KT2):
            emit_final(kk)
        b3_wrep_ctx = make_wrep_ctx(bpc - 1, inT_tiles[bpc - 1], wu,
                                    final_cb=emit_final)
        pending[0] = None
        b3_wrep_ctx()

        outsb = sb.tile([bpc, size], F32, name="outsb", tag="outsb")
        for n2 in range(NO):
            nc.scalar.activation(outsb[:, n2 * NB:(n2 + 1) * NB], pfs[n2][:],
                                 AF.Tanh)
        nc.scalar.dma_start(out[:], outsb[:])

    nc.compile()
    return nc


def build_in_maps(inputs):
    """Host-side packing: shard batch over cores, transpose/cast/pack
    inputs and weights into the exact device layouts, and precompute the
    tiny h_t-only projections in f32."""
    import ml_dtypes

    F8 = ml_dtypes.float8_e4m3
    F16 = np.float16

    x = np.asarray(inputs["inputs"], dtype=np.float32)
    W_p = np.asarray(inputs["W_p"], np.float32)
    v_p = np.asarray(inputs["v_p"], np.float32)
    W_a = np.asarray(inputs["W_a"], np.float32)
    U_a = np.asarray(inputs["U_a"], np.float32)
    v_a = np.asarray(inputs["v_a"], np.float32)
    W_att = np.asarray(inputs["W_att"], np.float32)

    # [B, S, H] -> fp8 -> [B, P, K8, 2, S] (partition-major, k-pair pairs)
    x8 = np.ascontiguousarray(
        x.astype(F8).transpose(0, 2, 1)
        .reshape(B, K8, 2, P, S).transpose(0, 3, 1, 2, 4))

    h_t = x[:, -1, :]                                   # [B, H] f32
    wt = h_t @ U_a                                      # [B, H]
    p_t = 1.0 / (1.0 + np.exp(-(np.tanh(h_t @ W_p) @ v_p))) * S  # [B, 1]

    wa8 = np.ascontiguousarray(
        W_a.astype(F8).reshape(K8, 2, P, H).transpose(2, 0, 1, 3))
    va8 = np.zeros((P, K8V, 2, 16), dtype=F8)
    va8[:, :, :, 0] = v_a[:, 0].reshape(K8V, 2, P).transpose(2, 0, 1).astype(F8)
    watt16 = np.ascontiguousarray(
        W_att.astype(F16).reshape(KT2, P, SIZE).transpose(1, 0, 2))

    in_maps = []
    for i in range(N_CORES):
        sl = slice(i * BPC, (i + 1) * BPC)
        wtT = np.ascontiguousarray(
            wt[sl].T.reshape(KT, P, BPC).transpose(1, 0, 2))
        comb0 = np.zeros((P, KT2, BPC), dtype=F16)
        comb0[:, KT:, :] = h_t[sl].T.reshape(KT, P, BPC).transpose(1, 0, 2)
        prow = np.ascontiguousarray(p_t[sl].T.astype(np.float32))
        in_maps.append({
            "x8": x8[sl], "wa8": wa8, "va8": va8, "wtT": wtT,
            "comb0": comb0, "prow": prow, "watt16": watt16,
        })
    return in_maps


def kernel(**inputs):
    global _compiled
    from concourse import bass_utils

    if _compiled is None:
        _compiled = _build()

    in_maps = build_in_maps(inputs)
    res = bass_utils.run_bass_kernel_spmd(_compiled, in_maps,
                                          list(range(N_CORES)))
    return np.concatenate([res.results[i]["out"] for i in range(N_CORES)],
                          axis=0).astype(np.float32)


# revision 12
# speedup vs baseline: 1.9957x; 1.0415x over previous
"""Trainium2 Bass kernel: Bahdanau local-p attention (B=32, S=2048, H=1024).

Sharding: data-parallel over batch. Each of the 8 cores processes B/8 = 4
batches end-to-end (weights replicated); no collectives.

Host-side prep (inside kernel(), numpy): inputs are cast to fp8-e4m3 and
packed into the exact SBUF layouts (partition-major, DoubleRow k-pair
interleave) in BOTH orientations — transposed [H, S] for the score matmul
and natural [S, H] for the context matmul — so the device does only
contiguous DMAs: no cast DMAs, no DMA transposes. The tiny h_t-only
projections (p_t, U_a h_t bias, the h_t half of the concat; 0.2% of
FLOPs) are precomputed on host in f32; all S-dimension work runs on
device.

Per-core dataflow (per batch):
  1. WH^T tiles [128, 512] = W_a-tile^T @ inT on PE, fp8 DoubleRow.
  2. tanh(WH^T + U_a h_t) on ACT (per-partition bias), fp8 out,
     hp-pairs interleaved for the DoubleRow v_a dot.
  3. score = v_a^T tanh(...) via M=1 fp8 DoubleRow matmuls (PE).
  4. softmax (minus-max) + gaussian window on DVE/ACT rows; the
     gaussian factor is computed at batch start (independent of score)
     so the post-score critical chain is short. Weights scaled by 2^16
     into fp8 range.
  5. context = w^T @ x on PE: weight row transposed on-PE into fp8
     columns (16-padded for the DoubleRow LDWEIGHTS step constraint),
     then fp8 DoubleRow matmuls against the natural-layout x copy,
     descaled on ACT, transposed back into combT. This whole block for
     batch b is emitted after batch b+1's first hp-groups so the
     in-order PE queue never waits on the softmax chain.
  6. final tanh([ctx, h_t] @ W_att) fp16 matmuls; the h_t half runs
     during batch 3's softmax, the ctx half interleaves with batch 3's
     context block.
"""

import math
from contextlib import ExitStack

import numpy as np

B, S, H, SIZE = 32, 2048, 1024, 1024
N_CORES = 8
BPC = B // N_CORES
P = 128
NB = 512
KT = H // P        # 8  k-tiles over H
K8 = H // (2 * P)  # 4  double-k-tiles (DoubleRow)
K8V = KT // 2      # 4  hp-pairs for the v_a dot
SQ = S // NB       # 4  s blocks
NT = S // P        # 16 s-chunks of 128
NT2 = S // (2 * P)  # 8 double-s-chunks (ctx DoubleRow)
KT2 = 2 * H // P   # 16 k-tiles over 2H (final projection)
NO = SIZE // NB    # 2  output blocks
WSCALE = 65536.0   # fp8 range scale for the softmax weights

_compiled = None


def _build(bpc=BPC, s=S, h=H, size=SIZE, debug=False):
    import concourse.bacc as bacc
    import concourse.mybir as mybir
    import concourse.tile as tile

    F32 = mybir.dt.float32
    F16 = mybir.dt.float16
    F8 = mybir.dt.float8e4
    AF = mybir.ActivationFunctionType
    ALU = mybir.AluOpType
    AX = mybir.AxisListType
    DR = mybir.MatmulPerfMode.DoubleRow

    denom = 2.0 * ((s // 2) / 2.0) ** 2
    inv_sq_denom = 1.0 / math.sqrt(denom)

    nc = bacc.Bacc("TRN2", target_bir_lowering=False, debug=debug)

    x8 = nc.dram_tensor("x8", [bpc, P, K8, 2, s], F8, kind="ExternalInput").ap()
    xn8 = nc.dram_tensor("xn8", [bpc, P, NT2, 2, h], F8,
                         kind="ExternalInput").ap()
    wa8 = nc.dram_tensor("wa8", [P, K8, 2, h], F8, kind="ExternalInput").ap()
    va8 = nc.dram_tensor("va8", [P, K8V, 2, 16], F8, kind="ExternalInput").ap()
    wtT_d = nc.dram_tensor("wtT", [P, KT, bpc], F32, kind="ExternalInput").ap()
    comb0 = nc.dram_tensor("comb0", [P, KT, bpc], F16,
                           kind="ExternalInput").ap()
    prow_d = nc.dram_tensor("prow", [1, bpc], F32, kind="ExternalInput").ap()
    watt16 = nc.dram_tensor("watt16", [P, KT2, size], F16,
                            kind="ExternalInput").ap()
    out = nc.dram_tensor("out", [bpc, size], F32, kind="ExternalOutput").ap()

    with tile.TileContext(nc) as tc, ExitStack() as ctx:
        sb = ctx.enter_context(tc.tile_pool(name="sb", bufs=1))
        ps = ctx.enter_context(tc.tile_pool(name="ps", bufs=1, space="PSUM"))

        inT_tiles = [None] * bpc
        xnt_tiles = [None] * bpc

        def emit_input_dma(b, chunked=False):
            inT = sb.tile([P, K8, 2, s], F8, name=f"inT_{b}", tag="big",
                          bufs=4)
            if chunked:
                for k2 in range(K8):
                    nc.sync.dma_start(inT[:, k2, :, :], x8[b, :, k2])
            else:
                nc.sync.dma_start(inT[:], x8[b])
            inT_tiles[b] = inT

        # ---- startup DMAs. Small tensors + weights on the ACT HWDGE
        # ring (wa chunked per k2 so hp0 can start on the first 256 KB);
        # the big inputs on the SP HWDGE ring.
        wtT = sb.tile([P, KT, bpc], F32, name="wtT", tag="wtT")
        nc.scalar.dma_start(wtT[:], wtT_d[:])
        wa_sb = sb.tile([P, K8, 2, h], F8, name="wa_sb", tag="wa")
        for k2 in range(K8):
            nc.scalar.dma_start(wa_sb[:, k2], wa8[:, k2])
        va_sb = sb.tile([P, K8V, 2, 16], F8, name="va_sb", tag="va")
        nc.scalar.dma_start(va_sb[:], va8[:])
        p_row = sb.tile([1, bpc], F32, name="p_row", tag="p_row")
        nc.scalar.dma_start(p_row[:], prow_d[:])
        emit_input_dma(0, chunked=True)
        emit_input_dma(1)
        emit_input_dma(2)
        emit_input_dma(3)
        for b in range(bpc):
            xnt = sb.tile([P, NT2, 2, h], F8, name=f"xnt_{b}", tag="bigN",
                          bufs=2)
            nc.sync.dma_start(xnt[:], xn8[b])
            xnt_tiles[b] = xnt

        # ---- constants / persistent tiles ----
        ident1 = sb.tile([1, 1], F32, name="ident1", tag="ident1")
        nc.vector.memset(ident1[:], 1.0)
        pos_row = sb.tile([1, s], F16, name="pos_row", tag="pos")
        nc.gpsimd.iota(pos_row[:], pattern=[[1, s]], base=0,
                       channel_multiplier=0,
                       allow_small_or_imprecise_dtypes=True)
        # fp8 weight columns, 16-padded; cols 1-15 stay zero forever
        wTp = sb.tile([P, NT2, 2, 16], F8, name="wTp", tag="wTp")
        nc.vector.memset(wTp[:], 0.0)
        combT = sb.tile([P, KT2, bpc], F16, name="combT", tag="combT")

        watt_holder = [None]

        def emit_watt_dma():
            w16 = sb.tile([P, KT2, size], F16, name="watt_sb", tag="wend")
            nc.scalar.dma_start(w16[:], watt16[:])
            watt_holder[0] = w16
            # h_t half of combT (ctx half written later by ctx blocks)
            nc.scalar.dma_start(combT[:, KT:, :], comb0[:])

        # deferred ctx-block emission for the previous batch
        pending = [None]

        # ---- main batch loop ----
        for b in range(bpc):
            if b == 1 or bpc <= 2:
                emit_watt_dma()
            inT = inT_tiles[b]

            # gaussian row: independent of the score — compute during
            # the batch's matmuls, off the post-score critical chain.
            dr = sb.tile([1, s], F16, name=f"dr_{b}", tag="gA")
            nc.vector.tensor_scalar(dr[:], pos_row[:], p_row[0:1, b:b + 1],
                                    inv_sq_denom, op0=ALU.subtract,
                                    op1=ALU.mult)
            d2 = sb.tile([1, s], F16, name=f"d2_{b}", tag="gB")
            nc.vector.tensor_mul(d2[:], dr[:], dr[:])
            gr = sb.tile([1, s], F16, name=f"gr_{b}", tag="gA")
            nc.scalar.activation(gr[:], d2[:], AF.Exp, scale=-1.0)

            # sc_ps allocated lazily at the first v_a matmul so the pool
            # rotation sequences them AFTER the deferred tiles of the
            # previous batch's ctx block.
            sc_ps = []

            def emit_va_mms(j, tanh_tiles):
                if j == 0:
                    sc_ps[:] = [ps.tile([16, NB], F32, name=f"sc_{b}_{q}",
                                        tag="sc", bufs=4) for q in range(SQ)]
                for q in range(SQ):
                    nc.tensor.matmul(sc_ps[q][:], va_sb[:, j],
                                     tanh_tiles[q][:],
                                     start=(j == 0), stop=(j == K8V - 1),
                                     perf_mode=DR, skip_group_check=True)

            # v_a matmuls run one hp-pair behind the main matmuls so the
            # PE never waits on ACT's tanh.
            pend2 = None
            for hp in range(KT):
                wh_ps = [ps.tile([P, NB], F32, name=f"wh_{b}_{hp}_{q}",
                                 tag="wh", bufs=4) for q in range(SQ)]
                for k2 in range(K8):
                    lhsT = wa_sb[:, k2, :, hp * P:(hp + 1) * P]
                    for q in range(SQ):
                        nc.tensor.matmul(
                            wh_ps[q][:], lhsT,
                            inT[:, k2, :, q * NB:(q + 1) * NB],
                            start=(k2 == 0), stop=(k2 == K8 - 1),
                            perf_mode=DR, skip_group_check=True)
                if hp == 2 and pending[0] is not None:
                    pending[0]()
                    pending[0] = None
                if hp % 2 == 0 and hp >= 2:
                    emit_va_mms(hp // 2 - 1, pend2)
                if hp % 2 == 0:
                    pend2 = [sb.tile([P, 2, NB], F8, name=f"th_{b}_{hp}_{q}",
                                     tag="tanh", bufs=8) for q in range(SQ)]
                for q in range(SQ):
                    nc.scalar.activation(pend2[q][:, hp % 2, :], wh_ps[q][:],
                                         AF.Tanh, bias=wtT[:, hp, b:b + 1])
            emit_va_mms(K8V - 1, pend2)

            # ---- softmax: short post-score chain ----
            score = sb.tile([1, s], F32, name=f"score_{b}", tag="score")
            for q in range(SQ):
                nc.vector.tensor_copy(score[0:1, q * NB:(q + 1) * NB],
                                      sc_ps[q][0:1, :])
            nmx = sb.tile([1, 1], F32, name=f"nmx_{b}", tag="nmx", bufs=2)
            nc.vector.tensor_reduce(nmx[:], score[:], axis=AX.X, op=ALU.max,
                                    negate=True)
            e1 = sb.tile([1, s], F16, name=f"e1_{b}", tag="e1")
            nc.scalar.activation(e1[:], score[:], AF.Exp, bias=nmx[0:1, 0:1])
            se = sb.tile([1, 1], F32, name=f"se_{b}", tag="se", bufs=2)
            nc.vector.tensor_reduce(se[:], e1[:], axis=AX.X, op=ALU.add)
            rr = sb.tile([1, 1], F32, name=f"rr_{b}", tag="rr", bufs=2)
            nc.vector.reciprocal(rr[:], se[:])
            rrS = sb.tile([1, 1], F32, name=f"rrS_{b}", tag="rrS", bufs=2)
            nc.vector.tensor_scalar_mul(rrS[:], rr[:], WSCALE)
            wu = sb.tile([1, s], F32, name=f"wu_{b}", tag="wu")
            nc.vector.scalar_tensor_tensor(wu[:], e1[:], rrS[0:1, 0:1], gr[:],
                                           op0=ALU.mult, op1=ALU.mult)

            def make_ctx(b, xnt, wu, final_cb=None):
                def emit():
                    # scaled weight row -> fp8 columns (on-PE transpose)
                    for t in range(NT):
                        pT = ps.tile([P, 1], F32, name=f"pT_{b}_{t}",
                                     tag="sc", bufs=4)
                        nc.tensor.transpose(pT[:],
                                            wu[0:1, t * P:(t + 1) * P],
                                            ident1[:])
                        nc.scalar.activation(wTp[:, t // 2, t % 2, 0:1],
                                             pT[:], AF.Copy)
                    # ctx row = w^T @ x, fp8 DoubleRow over s
                    pc = [ps.tile([16, NB], F32, name=f"pc_{b}_{hh}",
                                  tag="sc", bufs=4) for hh in range(2)]
                    for t2 in range(NT2):
                        lhsT = wTp[:, t2]
                        for hh in range(2):
                            nc.tensor.matmul(
                                pc[hh][:], lhsT,
                                xnt[:, t2, :, hh * NB:(hh + 1) * NB],
                                start=(t2 == 0), stop=(t2 == NT2 - 1),
                                perf_mode=DR, skip_group_check=True)
                    ctxrow = sb.tile([1, h], F32, name=f"ctxr_{b}",
                                     tag="ctxrow", bufs=2)
                    for hh in range(2):
                        nc.scalar.activation(
                            ctxrow[0:1, hh * NB:(hh + 1) * NB],
                            pc[hh][0:1, :], AF.Copy, scale=1.0 / WSCALE)
                    # back to column layout for the final projection
                    for kk in range(KT):
                        pC = ps.tile([P, 1], F32, name=f"pC_{b}_{kk}",
                                     tag="sc", bufs=4)
                        nc.tensor.transpose(pC[:],
                                            ctxrow[0:1, kk * P:(kk + 1) * P],
                                            ident1[:])
                        nc.vector.tensor_copy(combT[:, kk, b:b + 1], pC[:])
                        if final_cb is not None:
                            final_cb(kk)
                return emit

            pending[0] = make_ctx(b, xnt_tiles[b], wu)

        # ---- final projection: tanh([ctx, h_t] @ W_att) ----
        watt_sb = watt_holder[0]
        pfs = [ps.tile([bpc, NB], F32, name=f"pf_{n2}", tag="wh", bufs=4)
               for n2 in range(NO)]

        def emit_final(kk):
            for n2 in range(NO):
                nc.tensor.matmul(pfs[n2][:], combT[:, kk, :],
                                 watt_sb[:, kk, n2 * NB:(n2 + 1) * NB],
                                 start=(kk == KT), stop=(kk == KT - 1),
                                 skip_group_check=True)

        # h_t half runs during batch 3's softmax; ctx half interleaves
        # with batch 3's context block.
        for kk in range(KT, KT2):
            emit_final(kk)
        b3_ctx = make_ctx(bpc - 1, xnt_tiles[bpc - 1], wu,
                          final_cb=emit_final)
        pending[0] = None
        b3_ctx()

        outsb = sb.tile([bpc, size], F32, name="outsb", tag="outsb")
        for n2 in range(NO):
            nc.scalar.activation(outsb[:, n2 * NB:(n2 + 1) * NB], pfs[n2][:],
                                 AF.Tanh)
        nc.scalar.dma_start(out[:], outsb[:])

    nc.compile()
    return nc


def build_in_maps(inputs):
    """Host-side packing: shard batch over cores, transpose/cast/pack
    inputs and weights into the exact device layouts, and precompute the
    tiny h_t-only projections in f32."""
    import ml_dtypes

    F8 = ml_dtypes.float8_e4m3
    F16 = np.float16

    x = np.asarray(inputs["inputs"], dtype=np.float32)
    W_p = np.asarray(inputs["W_p"], np.float32)
    v_p = np.asarray(inputs["v_p"], np.float32)
    W_a = np.asarray(inputs["W_a"], np.float32)
    U_a = np.asarray(inputs["U_a"], np.float32)
    v_a = np.asarray(inputs["v_a"], np.float32)
    W_att = np.asarray(inputs["W_att"], np.float32)

    xf8 = x.astype(F8)
    # transposed copy [B, P, K8, 2, S] (h partition-major, k-pair pairs)
    x8 = np.ascontiguousarray(
        xf8.transpose(0, 2, 1).reshape(B, K8, 2, P, S).transpose(0, 3, 1, 2, 4))
    # natural copy [B, P, NT2, 2, H] (s partition-major, s-pair pairs)
    xn8 = np.ascontiguousarray(
        xf8.reshape(B, NT2, 2, P, H).transpose(0, 3, 1, 2, 4))

    h_t = x[:, -1, :]                                   # [B, H] f32
    wt = h_t @ U_a                                      # [B, H]
    p_t = 1.0 / (1.0 + np.exp(-(np.tanh(h_t @ W_p) @ v_p))) * S  # [B, 1]

    wa8 = np.ascontiguousarray(
        W_a.astype(F8).reshape(K8, 2, P, H).transpose(2, 0, 1, 3))
    va8 = np.zeros((P, K8V, 2, 16), dtype=F8)
    va8[:, :, :, 0] = v_a[:, 0].reshape(K8V, 2, P).transpose(2, 0, 1).astype(F8)
    watt16 = np.ascontiguousarray(
        W_att.astype(F16).reshape(KT2, P, SIZE).transpose(1, 0, 2))

    in_maps = []
    for i in range(N_CORES):
        sl = slice(i * BPC, (i + 1) * BPC)
        wtT = np.ascontiguousarray(
            wt[sl].T.reshape(KT, P, BPC).transpose(1, 0, 2))
        comb0 = np.ascontiguousarray(
            h_t[sl].T.reshape(KT, P, BPC).transpose(1, 0, 2).astype(F16))
        prow = np.ascontiguousarray(p_t[sl].T.astype(np.float32))
        in_maps.append({
            "x8": x8[sl], "xn8": xn8[sl], "wa8": wa8, "va8": va8,
            "wtT": wtT, "comb0": comb0, "prow": prow, "watt16": watt16,
        })
    return in_maps


def kernel(**inputs):
    global _compiled
    from concourse import bass_utils

    if _compiled is None:
        _compiled = _build()

    in_maps = build_in_maps(inputs)
    res = bass_utils.run_bass_kernel_spmd(_compiled, in_maps,
                                          list(range(N_CORES)))
    return np.concatenate([res.results[i]["out"] for i in range(N_CORES)],
                          axis=0).astype(np.float32)
